# revision 125
# baseline (speedup 1.0000x reference)
"""Trainium2 Bass kernel for nn_GraphVertExtraLinModel.

Model (per sample n, GS=4 graph channels, M=64 nodes):
  layer: h <- max_g relu(G[n,g] @ (h @ W[g].T + b[g]))  (+ residual for l>=1)
  head:  out = relu(h @ lin1_w.T + lin1_b) @ lin2_w.T + lin2_b

Sharding: data-parallel over N=128 -> 16 samples per core, weights replicated.
No collectives needed (the max-aggregation is over GS inside each sample).

Per-core layout (tokens = 16*64 = 1024, tiled 8 x 128; h kept FEATURE-major):
  mp    [tok, (g, p)] = hT.T @ W       (layers>=1: split-fp8 DoubleRow:
                                        h@W = h8@W8 + h8@dW8 + dh8@W8, all
                                        e4m3 with power-of-2 scales, 6 DR
                                        matmuls of K=256. L0 stays f32r.)
  bias  L0: rides the mm1 PSUM group as a rank-1 fp8-DR matmul per g,
        [s|s/64] x [b8|db8] (a 2-digit fp8 bias row; f32r + DR mix fine in
        one accumulation group, DR + DR with a second tile config NaNs on
        hw). Layers>=1: g0..g2 ride the DVE ms adds; g3 rides ONE rank-1
        bf16 matmul per p-chunk in the G-matmul group (brow x colsum(G3)).
        Net: 512 fewer PE cyc/tile than the old per-(g>=2,p) rank-1 scheme.
  ms    [tok, p] = mp (+ b)            (PSUM->SBUF bf16: 3x DVE add, 1x Act
                                        copy - Act cannot add a free-dim
                                        bias. g3's mm1 runs FIRST so the Act
                                        copy lands before the relus queue up)
  xoT   [p, tok] = ms.T @ Gblk         (G-matmul: ms stationary, G moving
                                        bf16; output lands feature-major ->
                                        NO transposes anywhere; emitted one
                                        t-iteration late, BEFORE mm1[t], so
                                        the relu chain gets a head start)
  xr    = relu(xo)*inv bf16 on Act     (descale rides the relu scale)
  h'    = max-tree(xr) + h             (bf16 maxes on DVE 2x, residual add
                                        on Pool; h stored bf16)
  h8    = fp8(qs*h) on Pool (tt-mult vs a memset qs tile); dh8 =
        fp8(qs*h-h8) on DVE. The quant chain is deferred 4-5 iterations
        (pend2/pend3) - its consumers are a full layer away, and emitting
        it fresh would head-block the in-order engine queues (the tile
        scheduler inserts blocking sem waits; this was worth ~50us).
fp8 h-scales are calibrated at runtime from a 2-sample host forward pass
(power-of-2 scales, 8x margin) before the program is built.
G is pre-transposed + block-diag packed (2 samples per 128x128 tile) on host.
A junk warm-up matmul burst runs during the startup DMAs so the PE p-state
ramp (3us to full clock) completes before real work starts.
"""

import math
import os

import numpy as np
import ml_dtypes
from contextlib import ExitStack

import concourse.bass as bass
import concourse.tile as tile
from concourse import bacc, mybir
from concourse.bass_utils import run_bass_kernel_spmd
from concourse.alu_op_type import AluOpType

F32 = mybir.dt.float32
F32R = mybir.dt.float32r
BF16 = mybir.dt.bfloat16
F8 = mybir.dt.float8e4
RELU = mybir.ActivationFunctionType.Relu
COPY = mybir.ActivationFunctionType.Copy
DR = mybir.MatmulPerfMode.DoubleRow
E4M3 = ml_dtypes.float8_e4m3fn

# timing-ablation switches (TimelineSim experiments only; break numerics)
_ABL = set(os.environ.get("KABL", "").split(",")) - {""}

N_CORES = 8
N_FULL = 128
N_LOC = N_FULL // N_CORES   # 16 samples per core
GS = 4
M = 64
C_IN = 128
D = 512
L = 8
TOK = N_LOC * M             # 1024 tokens per core
NT = TOK // 128             # 8 token tiles
KD = D // 128               # 4 contraction tiles for D
D2 = 2 * D                  # paired g-channel width


def _build_program(qs, sw):
    """qs[l]: fp8 scale of the h produced by layer l (l=0..L-2);
    sw[l]: fp8 scale of layer l's weights (l=1..L-1, index l-1)."""
    nc = bacc.Bacc(
        "TRN2",
        target_bir_lowering=False,
        debug=False,
        enable_asserts=False,
        num_devices=N_CORES,
    )

    xT_d = nc.dram_tensor("xT", [C_IN, TOK], BF16, kind="ExternalInput").ap()
    g_d = nc.dram_tensor("gsb", [128, GS * NT * 128], BF16, kind="ExternalInput").ap()
    w0_d = nc.dram_tensor("w0", [128, GS * D], BF16, kind="ExternalInput").ap()
    w8_d = nc.dram_tensor("w8", [L - 1, 128, GS * KD * D], F8, kind="ExternalInput").ap()
    dw8_d = nc.dram_tensor(
        "dw8", [L - 1, 128, GS * KD * D], F8, kind="ExternalInput"
    ).ap()
    # g0..g2 biases (scaled into mm1-PSUM units), broadcast over partitions
    b_d = nc.dram_tensor("b", [L - 1, 128, 3 * D], BF16, kind="ExternalInput").ap()
    # 2-digit fp8 bias rows for the L0 rank-1 PSUM matmul (f32r + DR mixes
    # fine in one group; DR + DR with a second config NaNs on hw)
    b80_d = nc.dram_tensor("b80", [1, GS * 2 * D], F8, kind="ExternalInput").ap()
    # L0 ladder row (s, s/64)
    lad_d = nc.dram_tensor("lad", [1, 256], F8, kind="ExternalInput").ap()
    # layers >= 1, g3: bias as rank-1 in the G-matmul group (bf16, proven):
    # scaled bias row + G column sums
    br_d = nc.dram_tensor("brow", [1, (L - 1) * D], BF16, kind="ExternalInput").ap()
    grs_d = nc.dram_tensor("grs", [1, NT * 128], BF16, kind="ExternalInput").ap()
    l1w_d = nc.dram_tensor("lin1", [128, KD * 128], BF16, kind="ExternalInput").ap()
    l1b_d = nc.dram_tensor("lin1b", [128, 1], F32, kind="ExternalInput").ap()
    l2w_d = nc.dram_tensor("lin2", [128, 1], F32R, kind="ExternalInput").ap()
    out_d = nc.dram_tensor("out", [1, TOK], F32, kind="ExternalOutput").ap()

    with tile.TileContext(nc) as tc, ExitStack() as ctx:
        const = ctx.enter_context(tc.tile_pool(name="const", bufs=1))
        wpool = ctx.enter_context(tc.tile_pool(name="w", bufs=2))
        dwpool = ctx.enter_context(tc.tile_pool(name="dw", bufs=2))
        bpool = ctx.enter_context(tc.tile_pool(name="b", bufs=2))
        hpool = ctx.enter_context(tc.tile_pool(name="h", bufs=3))
        qpool = ctx.enter_context(tc.tile_pool(name="q", bufs=2))
        h8pool = ctx.enter_context(tc.tile_pool(name="h8", bufs=2))
        dh8pool = ctx.enter_context(tc.tile_pool(name="dh8", bufs=2))
        mspool = ctx.enter_context(tc.tile_pool(name="ms", bufs=10))
        mtmp = ctx.enter_context(tc.tile_pool(name="mt", bufs=10))
        upool = ctx.enter_context(tc.tile_pool(name="u", bufs=6))
        # per-g mp tiles (1 PSUM bank each, 4-deep ring)
        mpsum = ctx.enter_context(tc.tile_pool(name="mpsum", bufs=4, space="PSUM"))
        xpsum = ctx.enter_context(tc.tile_pool(name="xpsum", bufs=2, space="PSUM"))

        # startup DMAs: the cost model serializes ALL DMA through one
        # resource, so issue everything on one queue in EXACT consumption
        # order (mm1 g3-first inputs, L0 bias rows, then G tiles by t)
        xsb = const.tile([128, TOK], BF16, tag="xsb")
        nc.sync.dma_start(out=xsb[:, 0:256], in_=xT_d[:, 0:256])
        wsb0 = wpool.tile([128, GS * D], BF16, tag="w0")
        gsb = const.tile([128, GS * NT * 128], BF16, tag="gsb")
        b80 = const.tile([1, GS * 2 * D], F8, tag="b80")
        # g3 chunk first: the g3-first mm1 uses it
        nc.sync.dma_start(out=wsb0[:, 3 * D :], in_=w0_d[:, 3 * D :])
        nc.sync.dma_start(out=b80[:], in_=b80_d)
        nc.sync.dma_start(out=wsb0[:, 0:D], in_=w0_d[:, 0:D])
        nc.sync.dma_start(out=gsb[:, 0:1024], in_=g_d[:, 0:1024])
        nc.sync.dma_start(out=wsb0[:, D:D2], in_=w0_d[:, D:D2])
        nc.sync.dma_start(out=wsb0[:, D2 : 3 * D], in_=w0_d[:, D2 : 3 * D])
        nc.sync.dma_start(out=xsb[:, 256:TOK], in_=xT_d[:, 256:TOK])
        nc.sync.dma_start(out=gsb[:, 1024:2560], in_=g_d[:, 1024:2560])
        nc.sync.dma_start(out=gsb[:, 2560:], in_=g_d[:, 2560:])
        brsb = const.tile([1, (L - 1) * D], BF16, tag="brow")
        nc.sync.dma_start(out=brsb[:], in_=br_d)
        grsb = const.tile([1, NT * 128], BF16, tag="grs")
        nc.sync.dma_start(out=grsb[:], in_=grs_d)
        l1sb = const.tile([128, KD * 128], BF16, tag="l1w")
        l1b = const.tile([128, 1], F32, tag="l1b")
        l2sb = const.tile([128, 1], F32R, tag="l2w")
        osb = const.tile([1, TOK], F32, tag="osb")

        # 2-digit ladder row for the L0 bias matmul: columns 0..127 = s,
        # 128..255 = s/64 (powers of two, exact in e4m3)
        lad8 = const.tile([1, 256], F8, tag="lad8")
        nc.sync.dma_start(out=lad8[:], in_=lad_d)

        # PE p-state warm-up: junk matmuls while the startup DMAs land (the
        # ramp needs ~3us of continuous execution to reach full clock)
        junk_s = const.tile([128, 128], BF16, tag="junks")
        junk_m = const.tile([128, 512], BF16, tag="junkm")
        nc.vector.memset(junk_s[:], 0.0)
        nc.vector.memset(junk_m[:], 0.0)
        junk_w = const.tile([128, D2], BF16, tag="junkw")
        nc.vector.memset(junk_w[:], 0.0)
        junk_o = mpsum.tile([128, 512], F32, tag="mp")
        for _ in range(int(os.environ.get("KWARM", "9"))):
            nc.tensor.matmul(junk_o[:], junk_s[:], junk_m[:], start=True, stop=True)

        def bias_dr(mp, g):
            src = b80[0:1, g * 2 * D : (g + 1) * 2 * D]
            nc.tensor.matmul(
                mp[:],
                lad8[0:1, 0:256].rearrange("p (i m) -> p i m", i=2),
                src.rearrange("p (i o) -> p i o", i=2),
                start=False,
                stop=True,
                perf_mode=DR,
            )

        # Two-stage deferred work: stage 1 (G-matmuls + relu + max tree +
        # residual) runs one t-iteration late so the PE never waits on the
        # helper engines' ms copies; stage 2 (fp8 quant chain) runs TWO
        # iterations late and is emitted first, so the 4-engine dependency
        # chain of an iteration never blocks the next iteration's helpers.
        pend1 = []
        pend2 = []
        pend3 = []
        tail_uh = []

        def flush2(drain=False):
            # lag-4: by emission time the u -> hs -> h8 -> dh8 chain's inputs
            # are iterations old, so these ops never head-block a queue
            if not pend2 or (
                not drain and len(pend2) < int(os.environ.get("KLAG2", "4"))
            ):
                return
            layer, t, u, hs, h_prev, h8_new, dh8_new, qsb = pend2.pop(0)
            if u is not None:
                # deferred residual: h = u + h_prev (Pool)
                nc.gpsimd.tensor_tensor(
                    hs, u[:], h_prev[:, t * D : (t + 1) * D], op=AluOpType.add
                )
            if h8_new is None or "noquant" in _ABL:
                return
            # quant chain off DVE entirely: hq = qs*h on Pool (bf16; exact -
            # qs is a power of two and h is bf16), h8 = fp8(hq) cast on Act,
            # dh8 = hq - h8 subtract on Pool
            h8s = h8_new[:, t * D : (t + 1) * D]
            hsrc = junk_m[:] if "quantconst" in _ABL else hs
            nc.gpsimd.tensor_tensor(h8s, hsrc, qsb[:], op=AluOpType.mult)
            # dh8 one iteration later still, so DVE never head-blocks on the
            # Pool h8 product
            pend3.append((layer, dh8_new[:, t * D : (t + 1) * D], hsrc, h8s))

        def flush3(drain=False):
            if not pend3 or (not drain and len(pend3) < int(os.environ.get("KLAG3", "2"))):
                return
            layer, dh8s, hsrc, h8s = pend3.pop(0)
            nc.vector.scalar_tensor_tensor(
                dh8s, hsrc, float(qs[layer]), h8s,
                op0=AluOpType.mult, op1=AluOpType.subtract,
            )

        def flush1():
            if not pend1:
                return
            (layer, t, ms_list, h_new, h_prev, h8_new, dh8_new, inv, qsb) = (
                pend1.pop(0)
            )
            xos = []
            for half in range(2):          # halves: (g0|g1), (g2|g3)
                xo = xpsum.tile([128, D2], F32, tag="xo")
                for gi in range(2):
                    g = half * 2 + gi
                    for p in range(KD):
                        xslice = xo[:, gi * D + p * 128 : gi * D + (p + 1) * 128]
                        rank1 = layer > 0 and g == 3
                        if rank1:
                            # g3 bias as rank-1: brow[p] x colsum(G3)[tok]
                            nc.tensor.matmul(
                                xslice,
                                brsb[
                                    0:1,
                                    (layer - 1) * D + p * 128 : (layer - 1) * D
                                    + (p + 1) * 128,
                                ],
                                grsb[0:1, t * 128 : (t + 1) * 128],
                                start=True,
                                stop=False,
                            )
                        nc.tensor.matmul(
                            xslice,
                            ms_list[g][:, p * 128 : (p + 1) * 128],
                            gsb[:, (t * GS + g) * 128 : (t * GS + g + 1) * 128],
                            start=not rank1,
                            stop=True,
                        )
                xos.append(xo)
            # relu (descale rides the scale; relu commutes with the max tree)
            xrs = []
            for half in range(2):
                xr = mtmp.tile([128, D2], BF16, tag="mt")
                xsrc = junk_w[:] if "reluconst" in _ABL else xos[half][:]
                nc.scalar.activation(xr[:], xsrc, func=RELU, scale=inv)
                xrs.append(xr)
            m01 = mtmp.tile([128, D], BF16, tag="mt")
            nc.vector.tensor_tensor(
                m01[:], xrs[0][:, 0:D], xrs[0][:, D:D2], op=AluOpType.max
            )
            m23 = mtmp.tile([128, D], BF16, tag="mt")
            nc.vector.tensor_tensor(
                m23[:], xrs[1][:, 0:D], xrs[1][:, D:D2], op=AluOpType.max
            )
            hs = h_new[:, t * D : (t + 1) * D]
            if h_prev is None or "noresid" in _ABL:
                # L0: no residual -> u is h directly
                nc.vector.tensor_tensor(hs, m01[:], m23[:], op=AluOpType.max)
                pend2.append((layer, t, None, hs, None, h8_new, dh8_new, qsb))
            else:
                tail = layer == L - 1 and t == NT - 1
                u = upool.tile([128, D], BF16, tag="u")
                nc.vector.tensor_tensor(u[:], m01[:], m23[:], op=AluOpType.max)
                # residual add on Pool (DVE on the last flush: the head isn't
                # gated on a Q7 launch)
                eng = nc.vector if tail else nc.gpsimd
                eng.tensor_tensor(
                    hs, u[:], h_prev[:, t * D : (t + 1) * D], op=AluOpType.add
                )
                pend2.append((layer, t, None, hs, None, h8_new, dh8_new, qsb))

        def emit_head(h_tile):
            # pipelined head: all four lin1 blocks first, then lin2 blocks
            # (each p2 waits on its Act relu, which overlaps the later p1s);
            # per-block output DMA so the final transfer isn't one serial 4KB
            h3 = h_tile[:].rearrange("p (t k) -> p t k", t=NT)
            x1s = []
            for tb in range(4):
                p1 = mpsum.tile([128, 256], F32, tag="mp")
                for c in range(KD):
                    nc.tensor.matmul(
                        p1[:],
                        l1sb[:, c * 128 : (c + 1) * 128],
                        h3[:, tb * 2 : (tb + 1) * 2, c * 128 : (c + 1) * 128],
                        start=(c == 0),
                        stop=(c == KD - 1),
                    )
                x1 = mtmp.tile([128, 256], F32R, tag="mt")
                if tb == 3:
                    # last block: relu on DVE right behind the p1 matmuls -
                    # Act's queue would add ~750ns to the critical tail
                    nc.vector.tensor_scalar(
                        x1[:], p1[:], l1b[:], 0.0,
                        op0=AluOpType.add, op1=AluOpType.max,
                    )
                else:
                    nc.scalar.activation(x1[:], p1[:], func=RELU, bias=l1b[:])
                x1s.append(x1)
            for tb in range(4):
                p2 = xpsum.tile([1, 256], F32, tag="xo")
                nc.tensor.matmul(p2[:], l2sb[:], x1s[tb][:], start=True, stop=True)
                osl = slice(tb * 256, (tb + 1) * 256)
                nc.vector.tensor_copy(osb[0:1, osl], p2[:])
                # SP queue for all chunks: Act's sequencer is busy with
                # the tail relus and would delay the issue
                nc.sync.dma_start(out=out_d[0:1, osl], in_=osb[0:1, osl])

        h_prev = None
        h8_prev = dh8_prev = None
        for layer in range(L):
            if layer == 0:
                wsb = wsb0
                dwsb = bsb = None
                inv = 1.0
            else:
                # per-g chunks so the first mm1 of the layer isn't gated on
                # the full weight transfer
                wsb = wpool.tile([128, GS * KD * D], F8, tag="w8")
                dwsb = dwpool.tile([128, GS * KD * D], F8, tag="dw8")
                # g3 first: mm1 consumes it first
                for g in (3, 0, 1, 2):
                    sl = slice(g * KD * D, (g + 1) * KD * D)
                    nc.sync.dma_start(out=wsb[:, sl], in_=w8_d[layer - 1][:, sl])
                    nc.sync.dma_start(out=dwsb[:, sl], in_=dw8_d[layer - 1][:, sl])
                bsb = bpool.tile([128, 3 * D], BF16, tag="b")
                nc.sync.dma_start(out=bsb[:], in_=b_d[layer - 1])
                inv = 1.0 / (qs[layer - 1] * sw[layer - 1])
            if layer == 1:
                # head weights: needed only at the very end
                nc.sync.dma_start(out=l1sb[:], in_=l1w_d)
                nc.sync.dma_start(out=l1b[:], in_=l1b_d)
                nc.sync.dma_start(out=l2sb[:], in_=l2w_d)

            h_new = hpool.tile([128, NT * D], BF16, tag="h")
            if layer < L - 1:
                h8_new = h8pool.tile([128, NT * D], F8, tag="h8")
                dh8_new = dh8pool.tile([128, NT * D], F8, tag="dh8")
                qsb = qpool.tile([128, D], BF16, tag="qs")
                nc.gpsimd.memset(qsb[:], float(qs[layer]))
            else:
                h8_new = dh8_new = qsb = None
            for t in range(NT):
                # PE program order: G-matmuls of t-1 FIRST (their ms inputs
                # completed last iteration), so the relu->max->residual chain
                # gets a full mm1-block head start; quant of t-2 goes LAST so
                # it never blocks ready work at an engine queue head.
                flush1()
                ms_list = [None] * GS
                # g3 first: its ms is the Act copy, and Act must get it done
                # before the relus of t-1 land in its queue (otherwise the
                # whole relu->xo-ring chain slips by an mm1 block)
                for g in (3, 0, 1, 2):
                    mp = mpsum.tile([128, D], F32, tag="mp")
                    if layer == 0:
                        nc.tensor.matmul(
                            mp[:],
                            xsb[:, t * 128 : (t + 1) * 128],
                            wsb[:, g * D : (g + 1) * D],
                            start=True,
                            stop=False,
                        )
                        bias_dr(mp, g)
                    else:
                        # split-fp8: h8@W8 + h8@dW8 + dh8@W8, two K=256
                        # DoubleRow passes each (q selects c-tile pair)
                        terms = (
                            (h8_prev, wsb),
                            (h8_prev, dwsb),
                            (dh8_prev, wsb),
                        )
                        for ti, (hsrc, wsrc) in enumerate(terms):
                            for q in range(2):
                                lhs = hsrc[
                                    :,
                                    t * D + 2 * q * 128 : t * D + (2 * q + 2) * 128,
                                ].rearrange("p (i m) -> p i m", i=2)
                                rhs = wsrc[
                                    :, (g * 2 + q) * 1024 : (g * 2 + q + 1) * 1024
                                ].rearrange("p (i o) -> p i o", i=2)
                                nc.tensor.matmul(
                                    mp[:],
                                    lhs,
                                    rhs,
                                    start=(ti == 0 and q == 0),
                                    stop=(ti == 2 and q == 1),
                                    perf_mode=DR,
                                )
                    ms = mspool.tile([128, D], BF16, tag="ms")
                    if layer == 0:
                        # bias rode the PSUM group; pure copies (Act carries
                        # the two relus, so only g3 goes there)
                        if g < 3:
                            nc.vector.tensor_copy(ms[:], mp[:])
                        else:
                            nc.scalar.activation(ms[:], mp[:], func=COPY)
                    elif g < 3:
                        # bias (host pre-scaled by qs*sw) rides the copy
                        nc.vector.tensor_tensor(
                            ms[:], mp[:], bsb[:, g * D : (g + 1) * D],
                            op=AluOpType.add,
                        )
                    else:
                        # bias rode the PSUM group via the rank-1 fp8 matmul
                        nc.scalar.activation(ms[:], mp[:], func=COPY)
                    ms_list[g] = ms
                flush3()
                flush2()
                pend1.append(
                    (layer, t, ms_list, h_new, h_prev, h8_new, dh8_new, inv, qsb)
                )
            h_prev = h_new
            h8_prev, dh8_prev = h8_new, dh8_new
        flush1()
        emit_head(h_prev)
        while pend2:
            flush2(drain=True)
        while pend3:
            flush3(drain=True)

    nc.compile()
    return nc


_NC = None


def _get_nc(qs=None, sw=None):
    global _NC
    if _NC is None:
        assert qs is not None and sw is not None
        _NC = _build_program(qs, sw)
    return _NC


def _pow2_scale(maxabs, margin):
    return 2.0 ** math.floor(math.log2(448.0 / (maxabs * margin)))


def _calibrate(G, x, W0, b0, W, b, n_samples=2):
    """Host forward pass on a couple of samples -> per-layer h max-abs."""
    Gs = G[:n_samples]
    h = x[:n_samples]
    qs = []
    for layer in range(L - 1):
        if layer == 0:
            Wl, bl = W0, b0
        else:
            Wl, bl = W[layer - 1], b[layer - 1]
        multi = np.einsum("nmc,gpc->gnmp", h, Wl, optimize=True) + bl[:, None, None, :]
        xo = np.einsum("ngij,gnjp->ngip", Gs, multi, optimize=True)
        hnew = np.maximum(xo, 0.0).max(axis=1)
        h = hnew + h if layer > 0 else hnew
        qs.append(_pow2_scale(np.abs(h).max(), 8.0))
    return qs


def _bias_scale(maxabs):
    """Power-of-two s so bias/s fits e4m3 (scaled biases reach ~4e3); s and
    s/64 must both be e4m3-representable, so clamp s to >= 2^-2."""
    return 2.0 ** max(math.ceil(math.log2(max(maxabs, 1e-30) / 224.0)), -2)


def _bias_digits(bvec, s):
    """2-digit fp8 row for the rank-1 PSUM bias: b ~= s*b8 + (s/64)*db8."""
    bvec = np.asarray(bvec, dtype=np.float32)
    b8 = (bvec / s).astype(E4M3)
    db8 = ((bvec / s - b8.astype(np.float32)) * 64.0).astype(E4M3)
    return b8, db8


def _prep_in_maps(G, x, W0, b0, W, b, lin1_w, lin1_b, lin2_w, lin2_b, sw, qs):
    BF = ml_dtypes.bfloat16
    G = np.ascontiguousarray(np.asarray(G, dtype=np.float32))
    x = np.ascontiguousarray(np.asarray(x, dtype=np.float32))
    W0 = np.asarray(W0, dtype=np.float32)
    b0 = np.asarray(b0, dtype=np.float32)
    W = np.asarray(W, dtype=np.float32)
    b = np.asarray(b, dtype=np.float32)
    lin1_w = np.asarray(lin1_w, dtype=np.float32)
    lin1_b = np.asarray(lin1_b, dtype=np.float32)
    lin2_w = np.asarray(lin2_w, dtype=np.float32)

    # w0: [c_local, (g, p)] with row = input channel c (bf16, layer 0:
    # halves the startup DMA on the serialized transfer resource)
    w0f = np.ascontiguousarray(
        W0.transpose(2, 0, 1).reshape(C_IN, GS * D)
    ).astype(BF)
    # layers >= 1: split-fp8 in DoubleRow layout [l, pc, (g, q, i, pout)]
    # with contraction c = (2q + i)*128 + pc
    Wd = W.reshape(L - 1, GS, D, 2, 2, 128).transpose(0, 5, 1, 3, 4, 2)
    Wd = np.ascontiguousarray(Wd.reshape(L - 1, 128, GS * KD * D))
    w8 = np.empty_like(Wd, dtype=E4M3)
    dw8 = np.empty_like(Wd, dtype=E4M3)
    for l in range(L - 1):
        ws = Wd[l] * sw[l]
        w8[l] = ws.astype(E4M3)
        dw8[l] = (ws - w8[l].astype(np.float32)).astype(E4M3)
    # biases pre-scaled by the layer's fp8 scale product (descale fused in
    # the max-tree stt). g0..g2 broadcast rows for the DVE ms-adds:
    bscale = np.array([qs[l] * sw[l] for l in range(L - 1)], np.float32)
    bsc = b.reshape(L - 1, GS * D) * bscale[:, None]
    bf = np.ascontiguousarray(
        np.broadcast_to(
            bsc.reshape(L - 1, 1, GS * D)[:, :, : 3 * D], (L - 1, 128, 3 * D)
        )
    ).astype(BF)
    # L0: 2-digit fp8 bias rows ([g, i, o] layout) + ladder row (s, s/64)
    lad = np.zeros((1, 256), dtype=E4M3)
    s0 = _bias_scale(float(np.abs(b0).max()))
    lad[0, 0:128] = s0
    lad[0, 128:256] = s0 / 64.0
    b80 = np.zeros((1, GS * 2 * D), dtype=E4M3)
    for g in range(GS):
        b8g, db8g = _bias_digits(b0[g], s0)
        b80[0, g * 2 * D : g * 2 * D + D] = b8g
        b80[0, g * 2 * D + D : (g + 1) * 2 * D] = db8g
    # layers >= 1, g3: scaled bias row for the G-phase rank-1
    brow = np.ascontiguousarray(bsc[:, 3 * D :].reshape(1, (L - 1) * D)).astype(BF)
    # lin1: [c_local, (ctile, e)] bf16
    l1f = np.ascontiguousarray(
        lin1_w.T.reshape(KD, 128, 128).transpose(1, 0, 2).reshape(128, KD * 128)
    ).astype(BF)
    l1b = np.ascontiguousarray(lin1_b.reshape(128, 1))
    l2f = np.ascontiguousarray(lin2_w.T)  # [128, 1]

    in_maps = []
    for cix in range(N_CORES):
        Gc = G[cix * N_LOC : (cix + 1) * N_LOC]                      # [16,4,64,64]
        xs = x[cix * N_LOC : (cix + 1) * N_LOC]                      # [16,64,128]
        xT = np.ascontiguousarray(xs.reshape(TOK, C_IN).T).astype(BF)
        Gt = Gc.transpose(1, 0, 3, 2)                                # [4,16,64j,64i]
        gblk = np.zeros((GS, NT, 128, 128), np.float32)
        gblk[:, :, 0:64, 0:64] = Gt[:, 0::2]
        gblk[:, :, 64:128, 64:128] = Gt[:, 1::2]
        gf = np.ascontiguousarray(
            gblk.transpose(2, 1, 0, 3).reshape(128, NT * GS * 128)
        ).astype(BF)
        grs = np.ascontiguousarray(
            gblk[3].sum(axis=1).reshape(1, NT * 128)
        ).astype(BF)
        in_maps.append(
            {
                "xT": xT,
                "gsb": gf,
                "w0": w0f,
                "w8": w8,
                "dw8": dw8,
                "b": bf,
                "b80": b80,
                "brow": brow,
                "grs": grs,
                "lad": lad,
                "lin1": l1f,
                "lin1b": l1b,
                "lin2": l2f,
            }
        )

    return in_maps


def kernel(G, x, W0, b0, W, b, lin1_w, lin1_b, lin2_w, lin2_b, _trace=False):
    G = np.asarray(G, dtype=np.float32)
    x = np.asarray(x, dtype=np.float32)
    W = np.asarray(W, dtype=np.float32)
    W0 = np.asarray(W0, dtype=np.float32)
    b0 = np.asarray(b0, dtype=np.float32)
    b = np.asarray(b, dtype=np.float32)
    lin2_b = np.asarray(lin2_b, dtype=np.float32)
    sw = [_pow2_scale(np.abs(W[l]).max(), 2.0) for l in range(L - 1)]
    qs = _calibrate(G, x, W0, b0, W, b)
    in_maps = _prep_in_maps(
        G, x, W0, b0, W, b, lin1_w, lin1_b, lin2_w, lin2_b, sw, qs
    )
    res = run_bass_kernel_spmd(
        _get_nc(qs, sw), in_maps, list(range(N_CORES)), trace=_trace
    )
    kernel._last_results = res
    out = np.concatenate(
        [res.results[c]["out"].reshape(N_LOC, M, 1) for c in range(N_CORES)], axis=0
    )
    return (out + lin2_b[0]).astype(np.float32)
